# revision 17
# baseline (speedup 1.0000x reference)
"""Trainium2 Bass kernel for nn_DependencyParsing (embedding_lookup).

Strategy (pure data-parallel over 8 NeuronCores, B=65536 -> 8192/core):
  - word_table cast to bf16, rows padded to 256B; word embeddings gathered
    feature-major straight into SBUF via SWDGE transpose dma_gather
    (512 idx / instruction HW cap), cycled over 2 SWDGE queues (one
    queue's descriptor ring serializes at ~4.75us/gather; 3+ queues
    corrupt gather packets when HWDGE traffic runs concurrently).
  - pos/dep lookups use no gather at all: pe@Wp + de@Wd is computed as
    onehot @ proj, where proj[s*64+cls] = table_s[cls] @ W_s (built on
    device, 28 small matmuls) and the one-hot [128, 512] per (t) comes
    from a single DVE is_equal of host-replicated fp16 index rows against
    a per-partition iota. Slot s=pos_t on partitions 0..63, dep_t on
    64..127. The combined bias (bw+bp+bd) rides a constant-1 row (t=0,
    partition 63) with proj row 63 = bias.
  - h = x @ W as bf16 matmuls (14 K-blocks x 6 M-tiles per 512 chunk)
    accumulating f32 in PSUM; h^3 = Square(h)*h on ACT+DVE -> bf16.
  - logits = h3 @ Wo; softmax stays class-major (no transpose): ACT
    Exp(logits+bo) from PSUM (logits are tiny, so no max subtraction),
    partition-sum via a ones-vector matmul, DVE reciprocal, PE ones
    broadcast, DVE multiply. Output lands [93, B] and the host transposes.
  - The previous chunk's logits/sum/broadcast matmuls are interleaved
    between the current chunk's M-tiles so the PE never idles long enough
    for the HAM clock gate to re-throttle.
"""

import os

import numpy as np
import ml_dtypes

import concourse.bacc as bacc
import concourse.mybir as mybir
import concourse.tile as tile
from concourse.bass_utils import run_bass_kernel_spmd

B, T, D, H, V, NPOS, NDEP, OUT = 65536, 7, 100, 700, 32000, 50, 45, 93
NCORES = 8
B_CORE = B // NCORES
CHUNK = 512
P = 128
# M-tiles over the 700 output features of h
MT = [(0, 128), (128, 128), (256, 128), (384, 128), (512, 128), (640, 60)]
# K-blocks for logits: 700 h-features in 6 blocks of 128 (last 60)
LKB = [(0, 128), (128, 128), (256, 128), (384, 128), (512, 128), (640, 60)]
dt = mybir.dt
bf16 = ml_dtypes.bfloat16
NQ = int(os.environ.get("KERNEL_NQ", "2"))

_NC_CACHE = {}


def build_nc(b_core):
    n_chunks = b_core // CHUNK
    nc = bacc.Bacc(None, target_bir_lowering=False, num_swdge_queues=max(NQ, 2))
    with tile.TileContext(nc) as tc:
        with tc.tile_pool(name="dram", bufs=1, space="DRAM") as dram:
            word_tab = dram.tile([V + 1, 128], dt.bfloat16, kind="ExternalInput",
                                 name="word_tab", uniquify=False)
            widx_d = dram.tile([P, T * n_chunks * 32], dt.int16, kind="ExternalInput",
                               name="widx", uniquify=False)
            vidx_d = dram.tile([P, n_chunks * T * CHUNK], dt.float16,
                               kind="ExternalInput", name="vidx", uniquify=False)
            tabt_d = dram.tile([P, 14 * 64], dt.bfloat16, kind="ExternalInput",
                               name="tabT", uniquify=False)
            iota_d = dram.tile([P, 1], dt.float32, kind="ExternalInput",
                               name="iota64", uniquify=False)
            ww_d = dram.tile([P, T * H], dt.bfloat16, kind="ExternalInput",
                             name="w_word", uniquify=False)
            wpd_d = dram.tile([P, 2 * T * H], dt.bfloat16, kind="ExternalInput",
                              name="w_pd", uniquify=False)
            wo_d = dram.tile([P, 6 * 96], dt.bfloat16, kind="ExternalInput",
                             name="w_o", uniquify=False)
            bias_d = dram.tile([1, H], dt.bfloat16, kind="ExternalInput",
                               name="bias_row", uniquify=False)
            bo_d = dram.tile([P, 1], dt.float32, kind="ExternalInput",
                             name="bo_pad", uniquify=False)
            out_d = dram.tile([OUT, b_core], dt.float32, kind="ExternalOutput",
                              name="out", uniquify=False)

            with (
                tc.tile_pool(name="const", bufs=1) as const,
                tc.tile_pool(name="wg", bufs=3) as wg_pool,
                tc.tile_pool(name="vx", bufs=2) as vx_pool,
                tc.tile_pool(name="oh", bufs=2) as oh_pool,
                tc.tile_pool(name="sq", bufs=2) as sq_pool,
                tc.tile_pool(name="h3", bufs=2) as h3_pool,
                tc.tile_pool(name="exq", bufs=2) as ex_pool,
                tc.tile_pool(name="rcq", bufs=2) as rc_pool,
                tc.tile_pool(name="opq", bufs=2) as op_pool,
                tc.tile_pool(name="hps", bufs=1, space="PSUM") as hps_pool,
                tc.tile_pool(name="ltps", bufs=2, space="PSUM") as ltps_pool,
            ):
                ww_sb = const.tile([P, T * H], dt.bfloat16, name="ww_sb")
                nc.sync.dma_start(out=ww_sb[:], in_=ww_d[:])
                wpd_sb = const.tile([P, 2 * T * H], dt.bfloat16, name="wpd_sb")
                nc.sync.dma_start(out=wpd_sb[:], in_=wpd_d[:])
                wo_sb = const.tile([P, 6 * 96], dt.bfloat16, name="wo_sb")
                nc.sync.dma_start(out=wo_sb[:], in_=wo_d[:])
                widx_sb = const.tile([P, T * n_chunks * 32], dt.int16, name="widx_sb")
                nc.sync.dma_start(out=widx_sb[:], in_=widx_d[:])
                tabt_sb = const.tile([P, 14 * 64], dt.bfloat16, name="tabt_sb")
                nc.sync.dma_start(out=tabt_sb[:], in_=tabt_d[:])
                iota_sb = const.tile([P, 1], dt.float32, name="iota_sb")
                nc.sync.dma_start(out=iota_sb[:], in_=iota_d[:])
                bo_sb = const.tile([P, 1], dt.float32, name="bo_sb")
                nc.sync.dma_start(out=bo_sb[:], in_=bo_d[:])
                ones_col = const.tile([P, 1], dt.float32, name="ones_col")
                nc.vector.memset(ones_col[:, :], 1.0)
                ones_row = const.tile([1, 96], dt.float32, name="ones_row")
                nc.vector.memset(ones_row[:, :], 1.0)

                # ---- build proj[s*64+cls] = table_s[cls] @ W_s on device ----
                proj_sb = const.tile([P, T * H], dt.bfloat16, name="proj_sb")
                for t in range(T):
                    pp1 = ltps_pool.tile([P, 512], dt.float32, name="pp1", tag="lt")
                    pp2 = ltps_pool.tile([P, 188], dt.float32, name="pp2", tag="lt")
                    for half in range(2):
                        s = t * 2 + half
                        lhsT = tabt_sb[:, s * 64:(s + 1) * 64]
                        nc.tensor.matmul(pp1[64 * half:64 * half + 64, :], lhsT,
                                         wpd_sb[:, s * H:s * H + 512],
                                         start=True, stop=True)
                        nc.tensor.matmul(pp2[64 * half:64 * half + 64, :], lhsT,
                                         wpd_sb[:, s * H + 512:s * H + 700],
                                         start=True, stop=True)
                    nc.scalar.activation(proj_sb[:, t * H:t * H + 512], pp1[:, :],
                                         mybir.ActivationFunctionType.Copy)
                    nc.scalar.activation(proj_sb[:, t * H + 512:t * H + 700], pp2[:, :],
                                         mybir.ActivationFunctionType.Copy)
                # combined bias rides one-hot row 63 of tile t=0
                nc.sync.dma_start(out=proj_sb[63:64, 0:H], in_=bias_d[:, :])

                # Deferred epilogue pieces for the previous chunk.
                pend = {}

                def emit_logits(h3_list):
                    lg_ps = ltps_pool.tile([P, CHUNK], dt.float32, name="lg_ps", tag="lt")
                    for j, (k0, ksz) in enumerate(LKB):
                        nc.tensor.matmul(
                            lg_ps[:96, :],
                            wo_sb[:ksz, j * 96:(j + 1) * 96],
                            h3_list[j][:ksz, :],
                            start=(j == 0), stop=(j == 5),
                        )
                    ex = ex_pool.tile([P, CHUNK], dt.float32, name="ex")
                    nc.scalar.activation(ex[:96, :], lg_ps[:96, :],
                                         mybir.ActivationFunctionType.Exp,
                                         bias=bo_sb[:96, :])
                    pend["ex"] = ex

                def emit_sum():
                    sum_ps = ltps_pool.tile([P, CHUNK], dt.float32, name="sum_ps", tag="lt")
                    nc.tensor.matmul(sum_ps[:1, :], ones_col[:OUT, :],
                                     pend["ex"][:OUT, :], start=True, stop=True)
                    rc = rc_pool.tile([1, CHUNK], dt.float32, name="rc")
                    nc.vector.reciprocal(rc[:1, :], sum_ps[:1, :])
                    pend["rc"] = rc

                def emit_bcast(cc):
                    rcb_ps = ltps_pool.tile([P, CHUNK], dt.float32, name="rcb_ps", tag="lt")
                    nc.tensor.matmul(rcb_ps[:96, :], ones_row[:1, :96],
                                     pend["rc"][:1, :], start=True, stop=True)
                    opt = op_pool.tile([P, CHUNK], dt.float32, name="opt")
                    nc.vector.tensor_mul(opt[:96, :], pend["ex"][:96, :],
                                         rcb_ps[:96, :])
                    nc.sync.dma_start(out=out_d[:, cc * CHUNK:(cc + 1) * CHUNK],
                                      in_=opt[:OUT, :])

                qn = 0
                prev_h3 = None
                for c in range(n_chunks):
                    # ---- word gathers (feature-major), cycled over queues ----
                    wg = []
                    for t in range(T):
                        g = wg_pool.tile([P, CHUNK], dt.bfloat16, name=f"wg{t}")
                        nc.gpsimd.dma_gather(
                            g.rearrange("p (o n) -> p o n", o=1),
                            word_tab[:],
                            widx_sb[:, (t * n_chunks + c) * 32:(t * n_chunks + c + 1) * 32],
                            CHUNK, CHUNK, 128, transpose=True, queue_num=qn % NQ,
                        )
                        qn += 1
                        wg.append(g)

                    # ---- pos/dep one-hots from replicated fp16 idx rows ----
                    vx = vx_pool.tile([P, T * CHUNK], dt.float16, name="vx")
                    nc.sync.dma_start(
                        out=vx[:], in_=vidx_d[:, c * T * CHUNK:(c + 1) * T * CHUNK])
                    oh = []
                    for t in range(T):
                        o = oh_pool.tile([P, CHUNK], dt.bfloat16, name=f"oh{t}")
                        nc.vector.tensor_scalar(
                            o[:, :], vx[:, t * CHUNK:(t + 1) * CHUNK],
                            iota_sb[:, :], None, mybir.AluOpType.is_equal)
                        oh.append(o)

                    # ---- h = x @ W (+bias via one-hot row), h3 = h^2 * h ----
                    # Previous chunk's epilogue matmuls are interleaved between
                    # M-tiles so their cross-engine deps resolve off PE time.
                    h3 = []
                    for mi, (m0, msz) in enumerate(MT):
                        hp = hps_pool.tile([P, CHUNK], dt.float32, name=f"hps{mi}")
                        kb = 0
                        for t in range(T):
                            nc.tensor.matmul(
                                hp[:msz, :],
                                ww_sb[:, t * H + m0: t * H + m0 + msz],
                                wg[t][:, :],
                                start=(kb == 0), stop=(kb == 13),
                            )
                            kb += 1
                        for t in range(T):
                            nc.tensor.matmul(
                                hp[:msz, :],
                                proj_sb[:, t * H + m0: t * H + m0 + msz],
                                oh[t][:, :],
                                start=(kb == 0), stop=(kb == 13),
                            )
                            kb += 1
                        sq = sq_pool.tile([P, CHUNK], dt.float32, name="sq")
                        nc.scalar.square(sq[:msz, :], hp[:msz, :])
                        h3t = h3_pool.tile([P, CHUNK], dt.bfloat16, name=f"h3_{mi}")
                        nc.vector.tensor_mul(h3t[:msz, :], sq[:msz, :], hp[:msz, :])
                        h3.append(h3t)
                        if prev_h3 is not None:
                            if mi == 0:
                                emit_logits(prev_h3)
                            elif mi == 2:
                                emit_sum()
                            elif mi == 4:
                                emit_bcast(c - 1)
                    prev_h3 = h3

                # tail epilogue for the last chunk
                emit_logits(prev_h3)
                emit_sum()
                emit_bcast(n_chunks - 1)
    nc.compile()
    return nc


def _wrap_idx(idx_tc):
    """[CHUNK] -> [128, 32] wrapped (i -> [i%16, i//16]) + replicated x8."""
    n = idx_tc.shape[0]
    w = idx_tc.reshape(n // 16, 16).T  # [16, n/16]
    return np.tile(w, (8, 1))


def prep_inputs(word_idx, pos_idx, dep_idx, word_table, pos_table, dep_table,
                Ww, bw, Wp, bp, Wd, bd, Wo, bo, b_core):
    """Returns (shared_map, per_core_fn). Host work is layout-only + small."""
    n_chunks = b_core // CHUNK

    wt = np.zeros((V + 1, 128), dtype=bf16)
    wt[:V, :D] = np.asarray(word_table, np.float32).astype(bf16)

    # pos/dep tables transposed: tabT[p, s*64+cls] = table_s[cls, p]
    tabt = np.zeros((P, 14 * 64), dtype=bf16)
    pt = np.asarray(pos_table, np.float32).astype(bf16)
    dtab = np.asarray(dep_table, np.float32).astype(bf16)
    for t in range(T):
        tabt[:D, (2 * t) * 64:(2 * t) * 64 + NPOS] = pt.T
        tabt[:D, (2 * t + 1) * 64:(2 * t + 1) * 64 + NDEP] = dtab.T

    iota64 = (np.arange(P) % 64).astype(np.float32).reshape(P, 1)

    bias_all = (np.asarray(bw, np.float32) + np.asarray(bp, np.float32)
                + np.asarray(bd, np.float32))
    bias_row = bias_all.astype(bf16).reshape(1, H)

    def pack_w(Wmat):
        arr = np.zeros((T, P, H), dtype=bf16)
        Wmat = np.asarray(Wmat, np.float32)
        for t in range(T):
            arr[t, :D, :] = Wmat[D * t:D * (t + 1), :].astype(bf16)
        return arr

    ww = pack_w(Ww)
    wp = pack_w(Wp)
    wd = pack_w(Wd)
    wpd = np.zeros((T, 2, P, H), dtype=bf16)
    wpd[:, 0] = wp
    wpd[:, 1] = wd

    wo = np.zeros((6, P, 96), dtype=bf16)
    Wo32 = np.asarray(Wo, np.float32)
    for j, (k0, ksz) in enumerate(LKB):
        wo[j, :ksz, :OUT] = Wo32[k0:k0 + ksz, :].astype(bf16)

    bo_pad = np.zeros((P, 1), dtype=np.float32)
    bo_pad[:OUT, 0] = np.asarray(bo, np.float32)

    shared = {
        "word_tab": wt,
        "tabT": tabt,
        "iota64": iota64,
        "bias_row": bias_row,
        "w_word": np.ascontiguousarray(ww.transpose(1, 0, 2)).reshape(P, T * H),
        "w_pd": np.ascontiguousarray(wpd.transpose(2, 0, 1, 3)).reshape(P, 2 * T * H),
        "w_o": np.ascontiguousarray(wo.transpose(1, 0, 2)).reshape(P, 6 * 96),
        "bo_pad": bo_pad,
    }

    wi = np.asarray(word_idx, np.int64).copy()
    wi[wi < 0] = V
    wi = wi.astype(np.int16)
    pi16 = np.asarray(pos_idx, np.int32).astype(np.float16)
    di16 = np.asarray(dep_idx, np.int32).astype(np.float16)

    def core_map(core):
        s = slice(core * b_core, (core + 1) * b_core)
        wic = wi[s]
        widx = np.zeros((P, T, n_chunks, 32), dtype=np.int16)
        for t in range(T):
            for c in range(n_chunks):
                widx[:, t, c, :] = _wrap_idx(wic[c * CHUNK:(c + 1) * CHUNK, t])

        # vidx[p, c, t, i]: p<64 -> pos_idx, p>=64 -> dep_idx; (t=0, p=63) = 63
        pc = pi16[s].reshape(n_chunks, CHUNK, T).transpose(0, 2, 1)
        dc = di16[s].reshape(n_chunks, CHUNK, T).transpose(0, 2, 1)
        vidx = np.empty((P, n_chunks, T, CHUNK), dtype=np.float16)
        vidx[:64] = pc[None, :, :, :]
        vidx[64:] = dc[None, :, :, :]
        vidx[63, :, 0, :] = np.float16(63.0)

        m = dict(shared)
        m["widx"] = widx.reshape(P, T * n_chunks * 32)
        m["vidx"] = np.ascontiguousarray(vidx).reshape(P, n_chunks * T * CHUNK)
        return m

    return shared, core_map


def kernel(**inputs):
    b_core = B_CORE
    if b_core not in _NC_CACHE:
        _NC_CACHE[b_core] = build_nc(b_core)
    nc = _NC_CACHE[b_core]

    _, core_map = prep_inputs(b_core=b_core, **inputs)
    in_maps = [core_map(i) for i in range(NCORES)]
    res = run_bass_kernel_spmd(nc, in_maps, core_ids=list(range(NCORES)))
    out = np.concatenate([r["out"] for r in res.results], axis=1)  # [93, B]
    return np.ascontiguousarray(out.T).astype(np.float32)


# revision 18
# speedup vs baseline: 1.1962x; 1.1962x over previous
"""Trainium2 Bass kernel for nn_DependencyParsing (embedding_lookup).

Strategy (pure data-parallel over 8 NeuronCores, B=65536 -> 8192/core):
  - word_table cast to bf16, rows padded to 256B; word embeddings gathered
    feature-major straight into SBUF via SWDGE transpose dma_gather
    (512 idx / instruction HW cap), cycled over 2 SWDGE queues (one
    queue's descriptor ring serializes at ~4.75us/gather; 3+ queues
    corrupt gather packets when HWDGE traffic runs concurrently).
  - pos/dep lookups use no gather at all: pe@Wp + de@Wd is computed as
    onehot @ proj, where proj[s*64+cls] = table_s[cls] @ W_s (built on
    device, 28 small matmuls) and the one-hot [128, 512] per (t) comes
    from a single DVE is_equal of host-replicated fp16 index rows against
    a per-partition iota. Slot s=pos_t on partitions 0..63, dep_t on
    64..127. The combined bias (bw+bp+bd) rides a constant-1 row (t=0,
    partition 63) with proj row 63 = bias.
  - h = x @ W as bf16 matmuls (14 K-blocks x 6 M-tiles per 512 chunk)
    accumulating f32 in PSUM; h^3 = Square(h)*h on ACT+DVE -> bf16.
  - logits = h3 @ Wo; softmax stays class-major (no transpose): ACT
    Exp(logits+bo) from PSUM (logits are tiny, so no max subtraction),
    partition-sum via a ones-vector matmul, DVE reciprocal, PE ones
    broadcast, DVE multiply. Output lands [93, B] and the host transposes.
  - The previous chunk's logits/sum/broadcast matmuls are interleaved
    between the current chunk's M-tiles so the PE never idles long enough
    for the HAM clock gate to re-throttle.
"""

import os

import numpy as np
import ml_dtypes

import concourse.bacc as bacc
import concourse.mybir as mybir
import concourse.tile as tile
from concourse.bass_utils import run_bass_kernel_spmd

B, T, D, H, V, NPOS, NDEP, OUT = 65536, 7, 100, 700, 32000, 50, 45, 93
NCORES = 8
B_CORE = B // NCORES
CHUNK = 512
P = 128
# M-tiles over the 700 output features of h
MT = [(0, 128), (128, 128), (256, 128), (384, 128), (512, 128), (640, 60)]
# K-blocks for logits: 700 h-features in 6 blocks of 128 (last 60)
LKB = [(0, 128), (128, 128), (256, 128), (384, 128), (512, 128), (640, 60)]
dt = mybir.dt
bf16 = ml_dtypes.bfloat16
NQ = int(os.environ.get("KERNEL_NQ", "2"))

_NC_CACHE = {}


def build_nc(b_core):
    n_chunks = b_core // CHUNK
    nc = bacc.Bacc(None, target_bir_lowering=False, num_swdge_queues=max(NQ, 2))
    with tile.TileContext(nc) as tc:
        with tc.tile_pool(name="dram", bufs=1, space="DRAM") as dram:
            word_tab = dram.tile([V + 1, 128], dt.bfloat16, kind="ExternalInput",
                                 name="word_tab", uniquify=False)
            widx_d = dram.tile([P, T * n_chunks * 32], dt.int16, kind="ExternalInput",
                               name="widx", uniquify=False)
            vidx_d = dram.tile([P, n_chunks * T * CHUNK], dt.float16,
                               kind="ExternalInput", name="vidx", uniquify=False)
            tabt_d = dram.tile([P, 14 * 64], dt.bfloat16, kind="ExternalInput",
                               name="tabT", uniquify=False)
            iota_d = dram.tile([P, 1], dt.float32, kind="ExternalInput",
                               name="iota64", uniquify=False)
            ww_d = dram.tile([P, T * H], dt.bfloat16, kind="ExternalInput",
                             name="w_word", uniquify=False)
            wpd_d = dram.tile([P, 2 * T * H], dt.bfloat16, kind="ExternalInput",
                              name="w_pd", uniquify=False)
            wo_d = dram.tile([P, 6 * 96], dt.bfloat16, kind="ExternalInput",
                             name="w_o", uniquify=False)
            bias_d = dram.tile([1, H], dt.bfloat16, kind="ExternalInput",
                               name="bias_row", uniquify=False)
            bo_d = dram.tile([P, 1], dt.float32, kind="ExternalInput",
                             name="bo_pad", uniquify=False)
            out_d = dram.tile([OUT, b_core], dt.float32, kind="ExternalOutput",
                              name="out", uniquify=False)

            with (
                tc.tile_pool(name="const", bufs=1) as const,
                tc.tile_pool(name="wg", bufs=3) as wg_pool,
                tc.tile_pool(name="vx", bufs=3) as vx_pool,
                tc.tile_pool(name="oh", bufs=3) as oh_pool,
                tc.tile_pool(name="sq", bufs=3) as sq_pool,
                tc.tile_pool(name="h3", bufs=3) as h3_pool,
                tc.tile_pool(name="exq", bufs=3) as ex_pool,
                tc.tile_pool(name="rcq", bufs=2) as rc_pool,
                tc.tile_pool(name="opq", bufs=2) as op_pool,
                tc.tile_pool(name="hps", bufs=1, space="PSUM") as hps_pool,
                tc.tile_pool(name="ltps", bufs=2, space="PSUM") as ltps_pool,
            ):
                ww_sb = const.tile([P, T * H], dt.bfloat16, name="ww_sb")
                nc.sync.dma_start(out=ww_sb[:], in_=ww_d[:])
                wpd_sb = const.tile([P, 2 * T * H], dt.bfloat16, name="wpd_sb")
                nc.sync.dma_start(out=wpd_sb[:], in_=wpd_d[:])
                wo_sb = const.tile([P, 6 * 96], dt.bfloat16, name="wo_sb")
                nc.sync.dma_start(out=wo_sb[:], in_=wo_d[:])
                widx_sb = const.tile([P, T * n_chunks * 32], dt.int16, name="widx_sb")
                nc.sync.dma_start(out=widx_sb[:], in_=widx_d[:])
                tabt_sb = const.tile([P, 14 * 64], dt.bfloat16, name="tabt_sb")
                nc.sync.dma_start(out=tabt_sb[:], in_=tabt_d[:])
                iota_sb = const.tile([P, 1], dt.float32, name="iota_sb")
                nc.sync.dma_start(out=iota_sb[:], in_=iota_d[:])
                bo_sb = const.tile([P, 1], dt.float32, name="bo_sb")
                nc.sync.dma_start(out=bo_sb[:], in_=bo_d[:])
                ones_col = const.tile([P, 1], dt.float32, name="ones_col")
                nc.vector.memset(ones_col[:, :], 1.0)
                ones_row = const.tile([1, 96], dt.float32, name="ones_row")
                nc.vector.memset(ones_row[:, :], 1.0)

                # ---- build proj[s*64+cls] = table_s[cls] @ W_s on device ----
                proj_sb = const.tile([P, T * H], dt.bfloat16, name="proj_sb")
                for t in range(T):
                    pp1 = ltps_pool.tile([P, 512], dt.float32, name="pp1", tag="lt")
                    pp2 = ltps_pool.tile([P, 188], dt.float32, name="pp2", tag="lt")
                    for half in range(2):
                        s = t * 2 + half
                        lhsT = tabt_sb[:, s * 64:(s + 1) * 64]
                        nc.tensor.matmul(pp1[64 * half:64 * half + 64, :], lhsT,
                                         wpd_sb[:, s * H:s * H + 512],
                                         start=True, stop=True)
                        nc.tensor.matmul(pp2[64 * half:64 * half + 64, :], lhsT,
                                         wpd_sb[:, s * H + 512:s * H + 700],
                                         start=True, stop=True)
                    nc.scalar.activation(proj_sb[:, t * H:t * H + 512], pp1[:, :],
                                         mybir.ActivationFunctionType.Copy)
                    nc.scalar.activation(proj_sb[:, t * H + 512:t * H + 700], pp2[:, :],
                                         mybir.ActivationFunctionType.Copy)
                # combined bias rides one-hot row 63 of tile t=0
                nc.sync.dma_start(out=proj_sb[63:64, 0:H], in_=bias_d[:, :])

                # Deferred epilogue pieces for the previous chunk.
                pend = {}

                def emit_logits(h3_list):
                    lg_ps = ltps_pool.tile([P, CHUNK], dt.float32, name="lg_ps", tag="lt")
                    for j, (k0, ksz) in enumerate(LKB):
                        nc.tensor.matmul(
                            lg_ps[:96, :],
                            wo_sb[:ksz, j * 96:(j + 1) * 96],
                            h3_list[j][:ksz, :],
                            start=(j == 0), stop=(j == 5),
                        )
                    ex = ex_pool.tile([P, CHUNK], dt.float32, name="ex")
                    nc.scalar.activation(ex[:96, :], lg_ps[:96, :],
                                         mybir.ActivationFunctionType.Exp,
                                         bias=bo_sb[:96, :])
                    pend["ex"] = ex

                def emit_sum():
                    sum_ps = ltps_pool.tile([P, CHUNK], dt.float32, name="sum_ps", tag="lt")
                    nc.tensor.matmul(sum_ps[:1, :], ones_col[:OUT, :],
                                     pend["ex"][:OUT, :], start=True, stop=True)
                    rc = rc_pool.tile([1, CHUNK], dt.float32, name="rc")
                    nc.vector.reciprocal(rc[:1, :], sum_ps[:1, :])
                    pend["rc"] = rc

                def emit_bcast(cc):
                    rcb_ps = ltps_pool.tile([P, CHUNK], dt.float32, name="rcb_ps", tag="lt")
                    nc.tensor.matmul(rcb_ps[:96, :], ones_row[:1, :96],
                                     pend["rc"][:1, :], start=True, stop=True)
                    opt = op_pool.tile([P, CHUNK], dt.float32, name="opt")
                    nc.vector.tensor_mul(opt[:96, :], pend["ex"][:96, :],
                                         rcb_ps[:96, :])
                    nc.sync.dma_start(out=out_d[:, cc * CHUNK:(cc + 1) * CHUNK],
                                      in_=opt[:OUT, :])

                qn = 0
                prev_h3 = None
                for c in range(n_chunks):
                    # ---- word gathers (feature-major), cycled over queues ----
                    wg = []
                    for t in range(T):
                        g = wg_pool.tile([P, CHUNK], dt.bfloat16, name=f"wg{t}")
                        nc.gpsimd.dma_gather(
                            g.rearrange("p (o n) -> p o n", o=1),
                            word_tab[:],
                            widx_sb[:, (t * n_chunks + c) * 32:(t * n_chunks + c + 1) * 32],
                            CHUNK, CHUNK, 128, transpose=True, queue_num=qn % NQ,
                        )
                        qn += 1
                        wg.append(g)

                    # ---- pos/dep one-hots from replicated fp16 idx rows ----
                    vx = vx_pool.tile([P, T * CHUNK], dt.float16, name="vx")
                    nc.sync.dma_start(
                        out=vx[:], in_=vidx_d[:, c * T * CHUNK:(c + 1) * T * CHUNK])
                    oh = []
                    for t in range(T):
                        o = oh_pool.tile([P, CHUNK], dt.bfloat16, name=f"oh{t}")
                        nc.vector.tensor_scalar(
                            o[:, :], vx[:, t * CHUNK:(t + 1) * CHUNK],
                            iota_sb[:, :], None, mybir.AluOpType.is_equal)
                        oh.append(o)

                    # ---- h = x @ W (+bias via one-hot row), h3 = h^2 * h ----
                    # Previous chunk's epilogue matmuls are interleaved between
                    # M-tiles so their cross-engine deps resolve off PE time.
                    h3 = []
                    for mi, (m0, msz) in enumerate(MT):
                        hp = hps_pool.tile([P, CHUNK], dt.float32, name=f"hps{mi}")
                        kb = 0
                        for t in range(T):
                            nc.tensor.matmul(
                                hp[:msz, :],
                                ww_sb[:, t * H + m0: t * H + m0 + msz],
                                wg[t][:, :],
                                start=(kb == 0), stop=(kb == 13),
                            )
                            kb += 1
                        for t in range(T):
                            nc.tensor.matmul(
                                hp[:msz, :],
                                proj_sb[:, t * H + m0: t * H + m0 + msz],
                                oh[t][:, :],
                                start=(kb == 0), stop=(kb == 13),
                            )
                            kb += 1
                        sq = sq_pool.tile([P, CHUNK], dt.float32, name="sq")
                        nc.scalar.square(sq[:msz, :], hp[:msz, :])
                        h3t = h3_pool.tile([P, CHUNK], dt.bfloat16, name=f"h3_{mi}")
                        nc.vector.tensor_mul(h3t[:msz, :], sq[:msz, :], hp[:msz, :])
                        h3.append(h3t)
                        if prev_h3 is not None:
                            if mi == 0:
                                emit_logits(prev_h3)
                            elif mi == 2:
                                emit_sum()
                            elif mi == 4:
                                emit_bcast(c - 1)
                    prev_h3 = h3

                # tail epilogue for the last chunk
                emit_logits(prev_h3)
                emit_sum()
                emit_bcast(n_chunks - 1)
    nc.compile()
    return nc


def _wrap_idx(idx_tc):
    """[CHUNK] -> [128, 32] wrapped (i -> [i%16, i//16]) + replicated x8."""
    n = idx_tc.shape[0]
    w = idx_tc.reshape(n // 16, 16).T  # [16, n/16]
    return np.tile(w, (8, 1))


def prep_inputs(word_idx, pos_idx, dep_idx, word_table, pos_table, dep_table,
                Ww, bw, Wp, bp, Wd, bd, Wo, bo, b_core):
    """Returns (shared_map, per_core_fn). Host work is layout-only + small."""
    n_chunks = b_core // CHUNK

    wt = np.zeros((V + 1, 128), dtype=bf16)
    wt[:V, :D] = np.asarray(word_table, np.float32).astype(bf16)

    # pos/dep tables transposed: tabT[p, s*64+cls] = table_s[cls, p]
    tabt = np.zeros((P, 14 * 64), dtype=bf16)
    pt = np.asarray(pos_table, np.float32).astype(bf16)
    dtab = np.asarray(dep_table, np.float32).astype(bf16)
    for t in range(T):
        tabt[:D, (2 * t) * 64:(2 * t) * 64 + NPOS] = pt.T
        tabt[:D, (2 * t + 1) * 64:(2 * t + 1) * 64 + NDEP] = dtab.T

    iota64 = (np.arange(P) % 64).astype(np.float32).reshape(P, 1)

    bias_all = (np.asarray(bw, np.float32) + np.asarray(bp, np.float32)
                + np.asarray(bd, np.float32))
    bias_row = bias_all.astype(bf16).reshape(1, H)

    def pack_w(Wmat):
        arr = np.zeros((T, P, H), dtype=bf16)
        Wmat = np.asarray(Wmat, np.float32)
        for t in range(T):
            arr[t, :D, :] = Wmat[D * t:D * (t + 1), :].astype(bf16)
        return arr

    ww = pack_w(Ww)
    wp = pack_w(Wp)
    wd = pack_w(Wd)
    wpd = np.zeros((T, 2, P, H), dtype=bf16)
    wpd[:, 0] = wp
    wpd[:, 1] = wd

    wo = np.zeros((6, P, 96), dtype=bf16)
    Wo32 = np.asarray(Wo, np.float32)
    for j, (k0, ksz) in enumerate(LKB):
        wo[j, :ksz, :OUT] = Wo32[k0:k0 + ksz, :].astype(bf16)

    bo_pad = np.zeros((P, 1), dtype=np.float32)
    bo_pad[:OUT, 0] = np.asarray(bo, np.float32)

    shared = {
        "word_tab": wt,
        "tabT": tabt,
        "iota64": iota64,
        "bias_row": bias_row,
        "w_word": np.ascontiguousarray(ww.transpose(1, 0, 2)).reshape(P, T * H),
        "w_pd": np.ascontiguousarray(wpd.transpose(2, 0, 1, 3)).reshape(P, 2 * T * H),
        "w_o": np.ascontiguousarray(wo.transpose(1, 0, 2)).reshape(P, 6 * 96),
        "bo_pad": bo_pad,
    }

    wi = np.asarray(word_idx, np.int64).copy()
    wi[wi < 0] = V
    wi = wi.astype(np.int16)
    pi16 = np.asarray(pos_idx, np.int32).astype(np.float16)
    di16 = np.asarray(dep_idx, np.int32).astype(np.float16)

    def core_map(core):
        s = slice(core * b_core, (core + 1) * b_core)
        wic = wi[s]
        widx = np.zeros((P, T, n_chunks, 32), dtype=np.int16)
        for t in range(T):
            for c in range(n_chunks):
                widx[:, t, c, :] = _wrap_idx(wic[c * CHUNK:(c + 1) * CHUNK, t])

        # vidx[p, c, t, i]: p<64 -> pos_idx, p>=64 -> dep_idx; (t=0, p=63) = 63
        pc = pi16[s].reshape(n_chunks, CHUNK, T).transpose(0, 2, 1)
        dc = di16[s].reshape(n_chunks, CHUNK, T).transpose(0, 2, 1)
        vidx = np.empty((P, n_chunks, T, CHUNK), dtype=np.float16)
        vidx[:64] = pc[None, :, :, :]
        vidx[64:] = dc[None, :, :, :]
        vidx[63, :, 0, :] = np.float16(63.0)

        m = dict(shared)
        m["widx"] = widx.reshape(P, T * n_chunks * 32)
        m["vidx"] = np.ascontiguousarray(vidx).reshape(P, n_chunks * T * CHUNK)
        return m

    return shared, core_map


def kernel(**inputs):
    b_core = B_CORE
    if b_core not in _NC_CACHE:
        _NC_CACHE[b_core] = build_nc(b_core)
    nc = _NC_CACHE[b_core]

    _, core_map = prep_inputs(b_core=b_core, **inputs)
    in_maps = [core_map(i) for i in range(NCORES)]
    res = run_bass_kernel_spmd(nc, in_maps, core_ids=list(range(NCORES)))
    out = np.concatenate([r["out"] for r in res.results], axis=1)  # [93, B]
    return np.ascontiguousarray(out.T).astype(np.float32)
